# revision 8
# baseline (speedup 1.0000x reference)
"""Trainium2 Bass kernel for nn_AttentionFlowLayer (trilinear similarity).

Reference math (per batch b):
    S[t, j] = (H[t] * w3) . U[j]  +  H[t] . w1  +  U[j] . w2

Folded form used here: with U'[j, d] = w3[d] * U[j, d] + w1[d] and
s_u[j] = U[j] . w2,

    S^T[j, t] = sum_d U'[j, d] * H[t, d]  +  s_u[j]

so each 128x512 output tile of S^T needs ONE f32r matmul
(lhsT = U'^T chunk, rhs = H^T chunk) and the s_u bias is per-partition,
folded for free into the PSUM->SBUF copy (ScalarE activation-bias /
VectorE tensor_scalar add). The kernel writes S^T per batch; the host
transposes on gather.

Sharding: data-parallel over batch - 8 batches, one per NeuronCore.
Self-contained: hardcodes shapes B=8, T=J=2048, D=128, fp32.
"""

import numpy as np

import concourse.mybir as mybir
import concourse.tile as tile
from concourse import bacc
from concourse.bass_utils import run_bass_kernel_spmd
from concourse.masks import make_identity

F32 = mybir.dt.float32
F32R = mybir.dt.float32r
IDENT = mybir.ActivationFunctionType.Identity

B = 8          # batch -> one per core
T = 2048       # rows of S (t) and columns (j)
D = 128        # feature dim = contraction K
P = 128        # partitions / tile edge
NT = T // P    # 16 tiles per side
TCW = 512      # PSUM bank width in fp32 -> matmul N
NTC = T // TCW  # 4 t chunks per output row-block

U_LEAD = 3     # U-transpose lead distance ahead of the main loop

_NC_CACHE = {}


def _build_nc():
    nc = bacc.Bacc(
        "TRN2",
        target_bir_lowering=False,
        debug=False,
        num_devices=B,
    )
    H = nc.dram_tensor("H", [T, D], F32, kind="ExternalInput").ap()
    U = nc.dram_tensor("U", [T, D], F32, kind="ExternalInput").ap()
    w = nc.dram_tensor("weight", [3 * D], F32, kind="ExternalInput").ap()
    # Holds S^T for this batch; host transposes after gather.
    S = nc.dram_tensor("S", [T, T], F32, kind="ExternalOutput").ap()

    with tile.TileContext(nc) as tc:
        with (
            tc.tile_pool(name="persist", bufs=1) as pp,
            tc.tile_pool(name="tmp", bufs=3) as tmp,
            tc.tile_pool(name="psum_tr", bufs=2, space="PSUM") as psum_tr,
            tc.tile_pool(name="psum_sm", bufs=2, space="PSUM") as psum_sm,
            tc.tile_pool(name="psum_mm", bufs=4, space="PSUM") as psum_mm,
            tc.tile_pool(name="outp", bufs=3) as outp,
        ):
            ident = pp.tile([P, P], F32)
            make_identity(nc, ident[:])

            # Inputs, natural layout [p, ti, d] (t = ti*128 + p), chunked DMAs
            # so the first transposes can start early.
            H_sb = pp.tile([P, NT, D], F32)
            U_sb = pp.tile([P, NT, D], F32)
            H_r = H.rearrange("(ti p) d -> p ti d", p=P)
            U_r = U.rearrange("(ti p) d -> p ti d", p=P)
            # Split input loads across the two descriptor-gen paths (SWDGE on
            # gpsimd, HWDGE on sync) so they don't serialize on one sequencer.
            for c in range(2):
                csl = slice(8 * c, 8 * c + 8)
                nc.gpsimd.dma_start(out=U_sb[:, csl, :], in_=U_r[:, csl, :])
                nc.sync.dma_start(out=H_sb[:, csl, :], in_=H_r[:, csl, :])

            w_row = pp.tile([1, 3 * D], F32)
            nc.sync.dma_start(out=w_row[:], in_=w.unsqueeze(0))
            one_cell = pp.tile([1, 1], F32)
            nc.vector.memset(one_cell[:], 1.0)

            # weight columns [d, 1] via rank-1 matmuls (w_row_chunk^T x 1)
            wcol_ps = psum_sm.tile([P, 3], F32, tag="sv")
            for k in range(3):
                nc.tensor.matmul(
                    wcol_ps[:, k : k + 1],
                    w_row[0:1, k * D : (k + 1) * D],
                    one_cell[:],
                    start=True,
                    stop=True,
                )
            wcol = pp.tile([P, 3], F32)
            nc.scalar.copy(wcol[:], wcol_ps[:])
            w1col = wcol[:, 0:1]
            w2col = wcol[:, 1:2]
            w3col = wcol[:, 2:3]

            # Persistent transposed operands (d on partitions), f32r for PE
            HT = pp.tile([P, T], F32R)     # H^T
            UpT = pp.tile([P, T], F32R)    # U'^T = w3 * U^T + w1
            s_u_col = pp.tile([P, NT], F32)  # s_u, one 128-column per j-tile

            def do_h(ti):
                csl = slice(ti * P, (ti + 1) * P)
                h_ps = psum_tr.tile([P, P], F32, tag="tr")
                nc.tensor.transpose(h_ps[:], H_sb[:, ti, :], ident[:])
                if ti % 2 == 0:
                    nc.scalar.copy(HT[:, csl], h_ps[:])
                else:
                    nc.vector.tensor_copy(HT[:, csl], h_ps[:])

            def do_u(k):
                csl = slice(k * P, (k + 1) * P)
                u_ps = psum_tr.tile([P, P], F32, tag="tr")
                nc.tensor.transpose(u_ps[:], U_sb[:, k, :], ident[:])
                # U'^T chunk = w3 * U^T + w1, rounded to f32r
                nc.vector.tensor_scalar(
                    UpT[:, csl], u_ps[:], w3col, w1col,
                    op0=mybir.AluOpType.mult, op1=mybir.AluOpType.add,
                )
                # unscaled U^T chunk (fp32) for the s_u column
                ut_tmp = tmp.tile([P, P], F32, tag="ut")
                nc.scalar.copy(ut_tmp[:], u_ps[:])
                su_ps = psum_sm.tile([P, 1], F32, tag="sv")
                nc.tensor.matmul(su_ps[:], ut_tmp[:], w2col, start=True, stop=True)
                nc.scalar.copy(s_u_col[:, k : k + 1], su_ps[:])

            do_u(0)

            # Main loop: output row-block jt (128 j's x full t), 4 psum banks.
            # H transposes are folded into jt==0's chunk loop (chunk tc only
            # needs H tiles 4*tc..4*tc+3), so the first matmul starts early.
            for jt in range(NT):
                if jt + U_LEAD < NT:
                    do_u(jt + U_LEAD)
                jsl = slice(jt * P, (jt + 1) * P)
                su_b = s_u_col[:, jt : jt + 1]
                out_sb = outp.tile([P, T], F32)
                for tc_i in range(NTC):
                    if jt == 0:
                        for ti in range(4 * tc_i, 4 * tc_i + 4):
                            do_h(ti)
                        if tc_i == 0 and U_LEAD >= 2:
                            do_u(1)
                            do_u(2)
                    tsl = slice(tc_i * TCW, (tc_i + 1) * TCW)
                    ps = psum_mm.tile([P, TCW], F32, tag="mm")
                    nc.tensor.matmul(
                        ps[:], UpT[:, jsl], HT[:, tsl], start=True, stop=True
                    )
                    if tc_i % 2 == 0:
                        nc.scalar.activation(
                            out_sb[:, tsl], ps[:], IDENT, bias=su_b, scale=1.0
                        )
                    else:
                        nc.vector.tensor_scalar_add(out_sb[:, tsl], ps[:], su_b)
                    # Stream the output out in halves (quarters on the first
                    # and last row-blocks) so DMA starts early and drains fast.
                    if jt in (0, NT - 1):
                        nc.sync.dma_start(
                            out=S[jsl, tsl], in_=out_sb[:, tsl]
                        )
                    elif tc_i % 2 == 1:
                        hsl = slice((tc_i - 1) * TCW, (tc_i + 1) * TCW)
                        nc.sync.dma_start(out=S[jsl, hsl], in_=out_sb[:, hsl])

    nc.compile()
    return nc


def _get_nc():
    if "nc" not in _NC_CACHE:
        _NC_CACHE["nc"] = _build_nc()
    return _NC_CACHE["nc"]


def kernel_with_results(H, U, weight, trace=False):
    assert H.shape == (B, T, D) and U.shape == (B, T, D)
    assert weight.shape == (3 * D,)
    nc = _get_nc()
    in_maps = [
        {
            "H": np.ascontiguousarray(H[b], dtype=np.float32),
            "U": np.ascontiguousarray(U[b], dtype=np.float32),
            "weight": np.ascontiguousarray(weight, dtype=np.float32),
        }
        for b in range(B)
    ]
    res = run_bass_kernel_spmd(nc, in_maps, list(range(B)), trace=trace)
    # device output is S^T per batch
    out = np.stack([res.results[b]["S"].T for b in range(B)], axis=0)
    return out, res


def kernel(H, U, weight):
    out, _ = kernel_with_results(H, U, weight)
    return out


if __name__ == "__main__":
    rng = np.random.default_rng(0)
    H = rng.standard_normal((B, T, D)).astype(np.float32)
    U = rng.standard_normal((B, T, D)).astype(np.float32)
    w = rng.random(3 * D).astype(np.float32)
    out = kernel(H, U, w)
    print(out.shape, out.dtype)


# revision 12
# speedup vs baseline: 1.0786x; 1.0786x over previous
"""Trainium2 Bass kernel for nn_AttentionFlowLayer (trilinear similarity).

Reference math (per batch b):
    S[t, j] = (H[t] * w3) . U[j]  +  H[t] . w1  +  U[j] . w2

Folded form used here: with U'[j, d] = w3[d] * U[j, d] + w1[d] and
s_u[j] = U[j] . w2,

    S^T[j, t] = sum_d U'[j, d] * H[t, d]  +  s_u[j]

so each 128x512 output tile of S^T needs ONE f32r matmul
(lhsT = U'^T chunk, rhs = H^T chunk) and the s_u bias is per-partition,
folded for free into the PSUM->SBUF copy (ScalarE activation-bias /
VectorE tensor_scalar add). The kernel writes S^T per batch; the host
transposes on gather.

Sharding: data-parallel over batch - 8 batches, one per NeuronCore.
Self-contained: hardcodes shapes B=8, T=J=2048, D=128, fp32.
"""

import numpy as np

import concourse.mybir as mybir
import concourse.tile as tile
from concourse import bacc
from concourse.bass_utils import run_bass_kernel_spmd
from concourse.masks import make_identity

F32 = mybir.dt.float32
F32R = mybir.dt.float32r
IDENT = mybir.ActivationFunctionType.Identity

B = 8          # batch -> one per core
T = 2048       # rows of S (t) and columns (j)
D = 128        # feature dim = contraction K
P = 128        # partitions / tile edge
NT = T // P    # 16 tiles per side
TCW = 512      # PSUM bank width in fp32 -> matmul N
NTC = T // TCW  # 4 t chunks per output row-block

U_LEAD = 3     # U-transpose lead distance ahead of the main loop

_NC_CACHE = {}


def _build_nc():
    nc = bacc.Bacc(
        "TRN2",
        target_bir_lowering=False,
        debug=False,
        num_devices=B,
    )
    H = nc.dram_tensor("H", [T, D], F32, kind="ExternalInput").ap()
    U = nc.dram_tensor("U", [T, D], F32, kind="ExternalInput").ap()
    w = nc.dram_tensor("weight", [3 * D], F32, kind="ExternalInput").ap()
    # Holds S^T for this batch; host transposes after gather.
    S = nc.dram_tensor("S", [T, T], F32, kind="ExternalOutput").ap()

    with tile.TileContext(nc) as tc:
        with (
            tc.tile_pool(name="persist", bufs=1) as pp,
            tc.tile_pool(name="tmp", bufs=5) as tmp,
            tc.tile_pool(name="psum_tr", bufs=2, space="PSUM") as psum_tr,
            tc.tile_pool(name="psum_sm", bufs=2, space="PSUM") as psum_sm,
            tc.tile_pool(name="psum_mm", bufs=4, space="PSUM") as psum_mm,
            tc.tile_pool(name="outp", bufs=3) as outp,
        ):
            ident = pp.tile([P, P], F32)
            make_identity(nc, ident[:])

            # Inputs, natural layout [p, ti, d] (t = ti*128 + p), chunked DMAs
            # so the first transposes can start early.
            H_sb = pp.tile([P, NT, D], F32)
            U_sb = pp.tile([P, NT, D], F32)
            H_r = H.rearrange("(ti p) d -> p ti d", p=P)
            U_r = U.rearrange("(ti p) d -> p ti d", p=P)
            for c in range(2):
                csl = slice(8 * c, 8 * c + 8)
                nc.sync.dma_start(out=U_sb[:, csl, :], in_=U_r[:, csl, :])
                nc.sync.dma_start(out=H_sb[:, csl, :], in_=H_r[:, csl, :])

            w_row = pp.tile([1, 3 * D], F32)
            nc.sync.dma_start(out=w_row[:], in_=w.unsqueeze(0))
            one_cell = pp.tile([1, 1], F32)
            nc.vector.memset(one_cell[:], 1.0)

            # weight columns [d, 1] via rank-1 matmuls (w_row_chunk^T x 1)
            wcol_ps = psum_sm.tile([P, 3], F32, tag="sv")
            for k in range(3):
                nc.tensor.matmul(
                    wcol_ps[:, k : k + 1],
                    w_row[0:1, k * D : (k + 1) * D],
                    one_cell[:],
                    start=True,
                    stop=True,
                )
            wcol = pp.tile([P, 3], F32)
            nc.scalar.copy(wcol[:], wcol_ps[:])
            w1col = wcol[:, 0:1]
            w2col = wcol[:, 1:2]
            w3col = wcol[:, 2:3]

            # Persistent transposed operands (d on partitions), f32r for PE
            HT = pp.tile([P, T], F32R)     # H^T
            UpT = pp.tile([P, T], F32R)    # U'^T = w3 * U^T + w1
            s_u_col = pp.tile([P, NT], F32)  # s_u, one 128-column per j-tile

            def do_h(ti):
                csl = slice(ti * P, (ti + 1) * P)
                h_ps = psum_tr.tile([P, P], F32, tag="tr")
                nc.tensor.transpose(h_ps[:], H_sb[:, ti, :], ident[:])
                if ti % 2 == 0:
                    nc.scalar.copy(HT[:, csl], h_ps[:])
                else:
                    nc.vector.tensor_copy(HT[:, csl], h_ps[:])

            ut_tmps = {}

            def do_u_tr(k):
                csl = slice(k * P, (k + 1) * P)
                u_ps = psum_tr.tile([P, P], F32, tag="tr")
                nc.tensor.transpose(u_ps[:], U_sb[:, k, :], ident[:])
                # U'^T chunk = w3 * U^T + w1, rounded to f32r
                nc.vector.tensor_scalar(
                    UpT[:, csl], u_ps[:], w3col, w1col,
                    op0=mybir.AluOpType.mult, op1=mybir.AluOpType.add,
                )
                # unscaled U^T chunk (fp32) for the s_u column
                ut_tmp = tmp.tile([P, P], F32, tag="ut", name=f"ut_tmp{k}")
                nc.scalar.copy(ut_tmp[:], u_ps[:])
                ut_tmps[k] = ut_tmp

            def do_u_su(k):
                su_ps = psum_sm.tile([P, 1], F32, tag="sv")
                nc.tensor.matmul(
                    su_ps[:], ut_tmps.pop(k)[:], w2col, start=True, stop=True
                )
                nc.scalar.copy(s_u_col[:, k : k + 1], su_ps[:])

            do_u_tr(0)

            # Main loop: output row-block jt (128 j's x full t), 4 psum banks.
            # H transposes are folded into jt==0's chunk loop (chunk tc only
            # needs H tiles 4*tc..4*tc+3), so the first matmul starts early.
            for jt in range(NT):
                do_u_su(jt)
                if jt + U_LEAD < NT:
                    do_u_tr(jt + U_LEAD)
                jsl = slice(jt * P, (jt + 1) * P)
                su_b = s_u_col[:, jt : jt + 1]
                out_sb = outp.tile([P, T], F32)
                for tc_i in range(NTC):
                    if jt == 0:
                        for ti in range(4 * tc_i, 4 * tc_i + 4):
                            do_h(ti)
                        if tc_i == 3:
                            do_u_tr(1)
                            do_u_tr(2)
                    tsl = slice(tc_i * TCW, (tc_i + 1) * TCW)
                    ps = psum_mm.tile([P, TCW], F32, tag="mm")
                    nc.tensor.matmul(
                        ps[:], UpT[:, jsl], HT[:, tsl], start=True, stop=True
                    )
                    if tc_i % 2 == 0:
                        nc.scalar.activation(
                            out_sb[:, tsl], ps[:], IDENT, bias=su_b, scale=1.0
                        )
                    else:
                        nc.vector.tensor_scalar_add(out_sb[:, tsl], ps[:], su_b)
                    # First row-block: stream quarters so DMA starts early;
                    # last row-block: halves to shorten the tail drain.
                    if jt == 0:
                        nc.sync.dma_start(out=S[jsl, tsl], in_=out_sb[:, tsl])
                    elif jt == NT - 1 and tc_i % 2 == 1:
                        hsl = slice((tc_i - 1) * TCW, (tc_i + 1) * TCW)
                        nc.sync.dma_start(out=S[jsl, hsl], in_=out_sb[:, hsl])
                if 0 < jt < NT - 1:
                    nc.sync.dma_start(out=S[jsl, :], in_=out_sb[:])

    nc.compile()
    return nc


def _get_nc():
    if "nc" not in _NC_CACHE:
        _NC_CACHE["nc"] = _build_nc()
    return _NC_CACHE["nc"]


def kernel_with_results(H, U, weight, trace=False):
    assert H.shape == (B, T, D) and U.shape == (B, T, D)
    assert weight.shape == (3 * D,)
    nc = _get_nc()
    in_maps = [
        {
            "H": np.ascontiguousarray(H[b], dtype=np.float32),
            "U": np.ascontiguousarray(U[b], dtype=np.float32),
            "weight": np.ascontiguousarray(weight, dtype=np.float32),
        }
        for b in range(B)
    ]
    res = run_bass_kernel_spmd(nc, in_maps, list(range(B)), trace=trace)
    # device output is S^T per batch
    out = np.stack([res.results[b]["S"].T for b in range(B)], axis=0)
    return out, res


def kernel(H, U, weight):
    out, _ = kernel_with_results(H, U, weight)
    return out


if __name__ == "__main__":
    rng = np.random.default_rng(0)
    H = rng.standard_normal((B, T, D)).astype(np.float32)
    U = rng.standard_normal((B, T, D)).astype(np.float32)
    w = rng.random(3 * D).astype(np.float32)
    out = kernel(H, U, w)
    print(out.shape, out.dtype)
